# revision 45
# baseline (speedup 1.0000x reference)
"""Causal multi-head attention (B=4, S=2048, D=1024, H=16, dk=64) on 8 TRN2
NeuronCores.

Sharding: core c = (batch b = c // 2, head-group g = c % 2 of 8 heads).
Each core computes, for its batch and its 8 heads:
    Q.T, K.T (feature-major) and V (seq-major) projections,
    S.T = K_h @ Q_h.T tiles (keys on partitions, queries on free dim),
    causal mask (additive -1e30 on the 128-wide diagonal band),
    exp (scale 1/sqrt(dk) folded into the ACT activation),
    A.T = [V_h | ones].T @ expS.T accumulated in PSUM (row 64 = softmax
    denominator, obtained for free), normalization via DVE reciprocal +
    a K=1 ones-matmul partition broadcast,
    partial out.T = W_o_slice.T-chunks @ A.T  (summed on host across the
    2 head-group cores of each batch).

All matmuls run as float32r (full PE speed for free dim >= 256; inputs are
fp32 bit patterns truncated to FP22 on read).
"""

import numpy as np

import concourse.bacc as bacc
import concourse.tile as tile
from concourse import mybir
from concourse import bass_utils

F32 = mybir.dt.float32
F32R = mybir.dt.float32r
P = 128          # partitions
S = 2048         # sequence length
D = 1024         # model dim
FLOC = 512       # local features per core (8 heads x 64)
HLOC = 8         # heads per core
DK = 64
NB = 4           # seq blocks of 512 (query blocks)
KC = 16          # key chunks of 128
DC = 8           # D chunks of 128
FC = 4           # local-feature chunks of 128 (= head pairs)
OB = 8           # output-D blocks of 128
NEG = -1.0e30
SCALE = 0.125    # 1 / sqrt(dk)

_CACHE = {}


def _emit(nc):
    xT = nc.dram_tensor("xT", [D, S], F32, kind="ExternalInput")
    wqT = nc.dram_tensor("wqT", [D, FLOC], F32, kind="ExternalInput")
    wkT = nc.dram_tensor("wkT", [D, FLOC], F32, kind="ExternalInput")
    wvT = nc.dram_tensor("wvT", [D, FLOC], F32, kind="ExternalInput")
    woT = nc.dram_tensor("woT", [FLOC, D], F32, kind="ExternalInput")
    onesc = nc.dram_tensor("onesc", [P, P], F32, kind="ExternalInput")
    maskb = nc.dram_tensor("maskb", [P, 2 * P], F32, kind="ExternalInput")
    outT = nc.dram_tensor("outT", [D, S], F32, kind="ExternalOutput")

    # DRAM APs in on-chip layouts
    xT_a = xT.ap().rearrange("(c p) s -> p c s", p=P).bitcast(F32R)      # [128, 8, 2048]
    wqT_a = wqT.ap().rearrange("(c p) f -> p c f", p=P).bitcast(F32R)    # [128, 8, 512]
    wkT_a = wkT.ap().rearrange("(c p) f -> p c f", p=P).bitcast(F32R)
    wvT_a = wvT.ap().rearrange("(c p) f -> p c f", p=P).bitcast(F32R)
    woT_a = woT.ap().rearrange("(c p) j -> p c j", p=P).bitcast(F32R)    # [128, 4, 1024]
    outT_a = outT.ap().rearrange("(c p) s -> p c s", p=P)                # [128, 8, 2048]

    with tile.TileContext(nc) as tc:
        import contextlib
        ctx = contextlib.ExitStack()
        with ctx:
            persist = ctx.enter_context(tc.tile_pool(name="persist", bufs=1))
            wpool = ctx.enter_context(tc.tile_pool(name="w", bufs=1))
            xtp = ctx.enter_context(tc.tile_pool(name="xt", bufs=1))
            qtp = ctx.enter_context(tc.tile_pool(name="qt", bufs=2))
            ep = ctx.enter_context(tc.tile_pool(name="e", bufs=6))
            atp = ctx.enter_context(tc.tile_pool(name="at", bufs=1))
            repp = ctx.enter_context(tc.tile_pool(name="rep", bufs=2))
            outp = ctx.enter_context(tc.tile_pool(name="out", bufs=2))
            invp = ctx.enter_context(tc.tile_pool(name="inv", bufs=1))
            pss = ctx.enter_context(tc.tile_pool(name="pss", bufs=2, space="PSUM"))
            pss2 = ctx.enter_context(tc.tile_pool(name="pss2", bufs=2, space="PSUM"))
            pav = ctx.enter_context(tc.tile_pool(name="pav", bufs=2, space="PSUM"))

            # static tiles; wq is split per D-chunk into separate tiles so
            # the first projection matmuls only wait on the pieces they read
            # (Tile tracks dependencies at whole-tile granularity).
            wq_sb = [wpool.tile([P, FLOC], F32R, tag=f"wq{dc}", name=f"wq{dc}")
                     for dc in range(DC)]
            wk_sb = wpool.tile([P, DC, FLOC], F32R, tag="wk")
            wv_sb = wpool.tile([P, DC, FLOC], F32R, tag="wv")
            wo_sb = persist.tile([P, FC, D], F32R, tag="wo")
            mask_sb = persist.tile([P, 2 * P], F32, tag="mask")
            ones1 = persist.tile([1, DK], F32R, tag="ones1")

            kt_blocks = []
            v_blocks = []
            qt_blocks = []

            def phase1_chunks(sb):
                """QKV projections for seq block sb, as a list of emission
                chunks so they can be interleaved with attention work."""
                chunks = []

                state = {}

                def setup():
                    xt_t = [xtp.tile([P, 512], F32R, tag=f"xt{dc}", name=f"xt{dc}")
                            for dc in range(DC)]
                    for dc in range(DC):
                        if sb == 0:
                            # interleave the wq pieces so the first projection
                            # group streams right behind the DMA dispatches
                            nc.sync.dma_start(wq_sb[dc][:], wqT_a[:, dc, :])
                        nc.sync.dma_start(
                            xt_t[dc][:],
                            xT_a[:, dc, sb * 512:(sb + 1) * 512])
                    qt_t = qtp.tile([P, FC, 512], F32R, tag="qt")
                    kt_t = persist.tile([P, FC, 512], F32R, tag=f"kt{sb}")
                    v_t = persist.tile([P, 4, HLOC, DK + 1], F32R, tag=f"v{sb}")
                    qt_blocks.append(qt_t)
                    kt_blocks.append(kt_t)
                    v_blocks.append(v_t)
                    # ones column of V_aug (softmax denominators)
                    nc.sync.dma_start(
                        v_t[:, :, :, DK],
                        onesc.ap()[:, 0:32].bitcast(F32R)
                        .rearrange("p (a b) -> p a b", a=4),
                    )
                    state.update(xt=xt_t, qt=qt_t, kt=kt_t, v=v_t)

                chunks.append(setup)

                def q_chunk(fc):
                    def run():
                        ps_q = pss.tile([P, 512], F32, tag="s")
                        for dc in range(DC):
                            nc.tensor.matmul(
                                ps_q[:], wq_sb[dc][:, fc * P:(fc + 1) * P],
                                state["xt"][dc][:],
                                start=(dc == 0), stop=(dc == DC - 1),
                            )
                        nc.scalar.copy(state["qt"][:, fc, :], ps_q[:])
                    return run

                def k_chunk(fc):
                    def run():
                        ps_k = pss.tile([P, 512], F32, tag="s")
                        for dc in range(DC):
                            nc.tensor.matmul(
                                ps_k[:], wk_sb[:, dc, fc * P:(fc + 1) * P],
                                state["xt"][dc][:],
                                start=(dc == 0), stop=(dc == DC - 1),
                            )
                        nc.scalar.copy(state["kt"][:, fc, :], ps_k[:])
                    return run

                def v_chunk(sc):
                    def run():
                        ps_v = pss.tile([P, 512], F32, tag="s")
                        for dc in range(DC):
                            nc.tensor.matmul(
                                ps_v[:], state["xt"][dc][:, sc * P:(sc + 1) * P],
                                wv_sb[:, dc, :],
                                start=(dc == 0), stop=(dc == DC - 1),
                            )
                        nc.scalar.copy(
                            state["v"][:, sc, :, 0:DK],
                            ps_v[:].rearrange("p (h d) -> p h d", h=HLOC),
                        )
                    return run

                for fc in range(FC):
                    chunks.append(q_chunk(fc))
                for fc in range(FC):
                    chunks.append(k_chunk(fc))
                for sc in range(4):
                    chunks.append(v_chunk(sc))
                return chunks

            def phase1(sb):
                for ch in phase1_chunks(sb):
                    ch()

            def attention_chunks(qb):
                """Attention + output projection for query block qb, as
                emission chunks [pair0..pair3, outproj]."""
                qt_t = qt_blocks[qb]
                at_t = atp.tile([P, FC, 512], F32R, tag="at")
                last = 4 * qb + 3

                def pair_chunk(p):
                    def run():
                        do_pair(p)
                    return run

                def do_pair(p):  # head pair
                    ps_a0 = pav.tile([DK + 1, 512], F32, tag="av")
                    ps_a1 = pav.tile([DK + 1, 512], F32, tag="av")

                    def emit_av(e_pair, col0, kc):
                        v_t = v_blocks[kc // 4]
                        nc.tensor.matmul(
                            ps_a0[:, col0:], v_t[:, kc % 4, 2 * p, :],
                            e_pair[:, 0, col0:], start=(kc == 0), stop=(kc == last),
                        )
                        nc.tensor.matmul(
                            ps_a1[:, col0:], v_t[:, kc % 4, 2 * p + 1, :],
                            e_pair[:, 1, col0:], start=(kc == 0), stop=(kc == last),
                        )

                    pend = []  # AV two cells behind: hides the exp latency
                    for kc in range(last + 1):
                        r = kc - 4 * qb
                        col0 = P * r if r >= 0 else 0
                        mcol0 = col0
                        kt_t = kt_blocks[kc // 4]
                        ks = slice((kc % 4) * P, (kc % 4 + 1) * P)
                        ps_s = pss2.tile([P, 2, 512], F32, tag="s2")
                        nc.tensor.matmul(
                            ps_s[:, 0, mcol0:], kt_t[0:DK, p, ks],
                            qt_t[0:DK, p, mcol0:], start=True, stop=True,
                        )
                        nc.tensor.matmul(
                            ps_s[:, 1, mcol0:], kt_t[DK:P, p, ks],
                            qt_t[DK:P, p, mcol0:], start=True, stop=True,
                        )
                        if r >= 0:
                            mstart = mcol0 if r == 3 else col0
                            region = slice(mstart, col0 + P)
                            w = col0 + P - mstart
                            nc.vector.tensor_tensor(
                                ps_s[:, :, region],
                                ps_s[:, :, region],
                                mask_sb[:, 2 * P - w:]
                                .unsqueeze(1).to_broadcast([P, 2, w]),
                                mybir.AluOpType.add,
                            )
                        e_pair = ep.tile([P, 2, 512], F32R, tag="e")
                        nc.scalar.activation(
                            e_pair[:, :, mcol0:], ps_s[:, :, mcol0:],
                            mybir.ActivationFunctionType.Exp, scale=SCALE,
                        )
                        pend.append((e_pair, mcol0, kc))
                        if len(pend) > 5:
                            emit_av(*pend.pop(0))
                    for it in pend:
                        emit_av(*it)
                    # normalize: at[f, q] = a[f, q] / a[64, q], heads 2p, 2p+1
                    for half, ps_a in ((0, ps_a0), (1, ps_a1)):
                        inv_r = invp.tile([1, 512], F32R, tag="invr")
                        with nc.allow_low_precision(reason="fp32r matmul input"):
                            nc.vector.reciprocal(inv_r[:], ps_a[DK:DK + 1, :])
                        ps_rep = pss.tile([P, 512], F32, tag="s")
                        nc.tensor.matmul(ps_rep[0:DK, :], ones1[:], inv_r[:],
                                         start=True, stop=True)
                        rep = repp.tile([DK, 512], F32, tag="rep")
                        if half == 0:
                            nc.scalar.copy(rep[:], ps_rep[0:DK, :])
                        else:
                            nc.vector.tensor_copy(rep[:], ps_rep[0:DK, :])
                        nc.vector.tensor_tensor(
                            at_t[half * DK:(half + 1) * DK, p, :],
                            ps_a[0:DK, :], rep[:], mybir.AluOpType.mult,
                        )
                def outproj():
                    # output projection: outT[j, q] partial
                    for ob in range(OB):
                        ps_o = pss.tile([P, 512], F32, tag="s")
                        for fc in range(FC):
                            nc.tensor.matmul(
                                ps_o[:], wo_sb[:, fc, ob * P:(ob + 1) * P],
                                at_t[:, fc, :],
                                start=(fc == 0), stop=(fc == FC - 1),
                            )
                        o_t = outp.tile([P, 512], F32, tag="o")
                        nc.vector.tensor_copy(o_t[:], ps_o[:])
                        nc.sync.dma_start(
                            outT_a[:, ob, qb * 512:(qb + 1) * 512], o_t[:])

                return [pair_chunk(p) for p in range(FC)], outproj

            # Emission: phase1(0) with the late weights DMA'd behind the
            # first projection groups, then attention(qb) interleaved with
            # phase1(qb+1) so attention's ACT-bound stretches overlap the
            # projections' PE-bound stretches.
            ph = phase1_chunks(0)
            ph[0]()                      # xt(0) DMA right after wq
            nc.sync.dma_start(wk_sb[:], wkT_a)
            nc.sync.dma_start(wv_sb[:], wvT_a)
            for ch in ph[1:5]:
                ch()
            nc.sync.dma_start(wo_sb[:], woT_a)
            nc.sync.dma_start(mask_sb[:], maskb.ap())
            nc.sync.dma_start(ones1[:], onesc.ap()[0:1, 0:DK].bitcast(F32R))
            for ch in ph[5:]:
                ch()
            for qb in range(NB):
                att, opj = attention_chunks(qb)
                att = att + [opj]
                nxt = phase1_chunks(qb + 1) if qb < NB - 1 else []
                seq = []
                k = 0
                if nxt:
                    seq.append(nxt[0])
                    k = 1
                for i, pc in enumerate(att):
                    seq.append(pc)
                    for _ in range(3):
                        if k < len(nxt):
                            seq.append(nxt[k])
                            k += 1
                while k < len(nxt):
                    seq.append(nxt[k])
                    k += 1
                for ch in seq:
                    ch()


def _build():
    nc = bacc.Bacc("TRN2", target_bir_lowering=False, debug=False)
    _emit(nc)
    nc.compile()
    return nc


def _make_in_maps(x, W_q, W_k, W_v, W_o):
    onesc = np.ones((P, P), dtype=np.float32)
    kk = np.arange(P)[:, None]
    jj = np.arange(P)[None, :]
    band = np.where(kk <= jj, 0.0, NEG).astype(np.float32)
    # [128, 256]: first 128 cols fully masked (r=3 widened tiles), then the
    # triangular diagonal band
    maskb = np.concatenate(
        [np.full((P, P), NEG, np.float32), band], axis=1)
    in_maps = []
    for c in range(8):
        b, g = divmod(c, 2)
        cols = slice(g * FLOC, (g + 1) * FLOC)
        in_maps.append({
            "xT": np.ascontiguousarray(x[b].T),
            "wqT": np.ascontiguousarray(W_q[cols, :].T),
            "wkT": np.ascontiguousarray(W_k[cols, :].T),
            "wvT": np.ascontiguousarray(W_v[cols, :].T),
            "woT": np.ascontiguousarray(W_o[:, cols].T),
            "onesc": onesc,
            "maskb": maskb,
        })
    return in_maps


def kernel(x, W_q, W_k, W_v, W_o):
    x = np.asarray(x, dtype=np.float32)
    W_q = np.asarray(W_q, dtype=np.float32)
    W_k = np.asarray(W_k, dtype=np.float32)
    W_v = np.asarray(W_v, dtype=np.float32)
    W_o = np.asarray(W_o, dtype=np.float32)
    if "nc" not in _CACHE:
        _CACHE["nc"] = _build()
    nc = _CACHE["nc"]
    in_maps = _make_in_maps(x, W_q, W_k, W_v, W_o)
    res = bass_utils.run_bass_kernel_spmd(nc, in_maps, core_ids=list(range(8)))
    B = x.shape[0]
    out = np.empty((B, S, D), dtype=np.float32)
    for b in range(B):
        acc = res.results[2 * b]["outT"] + res.results[2 * b + 1]["outT"]
        out[b] = acc.T
    return out


# revision 49
# speedup vs baseline: 1.3419x; 1.3419x over previous
"""Causal multi-head attention (B=4, S=2048, D=1024, H=16, dk=64) on 8 TRN2
NeuronCores.

Sharding: core c = (batch b = c // 2, head-group g = c % 2 of 8 heads).
Each core computes, for its batch and its 8 heads:
    Q.T, K.T (feature-major) and V (seq-major) projections,
    S.T = K_h @ Q_h.T tiles (keys on partitions, queries on free dim),
    causal mask (additive -1e30 on the 128-wide diagonal band),
    exp (scale 1/sqrt(dk) folded into the ACT activation),
    A.T = [V_h | ones].T @ expS.T accumulated in PSUM (row 64 = softmax
    denominator, obtained for free), normalization via DVE reciprocal +
    a K=1 ones-matmul partition broadcast,
    partial out.T = W_o_slice.T-chunks @ A.T  (summed on host across the
    2 head-group cores of each batch).

All matmuls run as float32r (full PE speed for free dim >= 256; inputs are
fp32 bit patterns truncated to FP22 on read).
"""

import numpy as np

import concourse.bacc as bacc
import concourse.tile as tile
from concourse import mybir
from concourse import bass_utils

F32 = mybir.dt.float32
F32R = mybir.dt.float32r
P = 128          # partitions
S = 2048         # sequence length
D = 1024         # model dim
FLOC = 512       # local features per core (8 heads x 64)
HLOC = 8         # heads per core
DK = 64
NB = 4           # seq blocks of 512 (query blocks)
KC = 16          # key chunks of 128
DC = 8           # D chunks of 128
FC = 4           # local-feature chunks of 128 (= head pairs)
OB = 8           # output-D blocks of 128
NEG = -1.0e30
SCALE = 0.125    # 1 / sqrt(dk)

_CACHE = {}


def _emit(nc):
    xT = nc.dram_tensor("xT", [D, S], F32, kind="ExternalInput")
    wqT = nc.dram_tensor("wqT", [D, FLOC], F32, kind="ExternalInput")
    wkT = nc.dram_tensor("wkT", [D, FLOC], F32, kind="ExternalInput")
    wvT = nc.dram_tensor("wvT", [D, FLOC], F32, kind="ExternalInput")
    woT = nc.dram_tensor("woT", [FLOC, D], F32, kind="ExternalInput")
    onesc = nc.dram_tensor("onesc", [P, P], F32, kind="ExternalInput")
    maskb = nc.dram_tensor("maskb", [P, 2 * P], F32, kind="ExternalInput")
    outT = nc.dram_tensor("outT", [D, S], F32, kind="ExternalOutput")

    # DRAM APs in on-chip layouts
    xT_a = xT.ap().rearrange("(c p) s -> p c s", p=P).bitcast(F32R)      # [128, 8, 2048]
    wqT_a = wqT.ap().rearrange("(c p) f -> p c f", p=P).bitcast(F32R)    # [128, 8, 512]
    wkT_a = wkT.ap().rearrange("(c p) f -> p c f", p=P).bitcast(F32R)
    wvT_a = wvT.ap().rearrange("(c p) f -> p c f", p=P).bitcast(F32R)
    woT_a = woT.ap().rearrange("(c p) j -> p c j", p=P).bitcast(F32R)    # [128, 4, 1024]
    outT_a = outT.ap().rearrange("(c p) s -> p c s", p=P)                # [128, 8, 2048]

    with tile.TileContext(nc) as tc:
        import contextlib
        ctx = contextlib.ExitStack()
        with ctx:
            persist = ctx.enter_context(tc.tile_pool(name="persist", bufs=1))
            wpool = ctx.enter_context(tc.tile_pool(name="w", bufs=1))
            xtp = ctx.enter_context(tc.tile_pool(name="xt", bufs=1))
            qtp = ctx.enter_context(tc.tile_pool(name="qt", bufs=2))
            ep = ctx.enter_context(tc.tile_pool(name="e", bufs=6))
            atp = ctx.enter_context(tc.tile_pool(name="at", bufs=1))
            repp = ctx.enter_context(tc.tile_pool(name="rep", bufs=2))
            outp = ctx.enter_context(tc.tile_pool(name="out", bufs=2))
            invp = ctx.enter_context(tc.tile_pool(name="inv", bufs=2))
            pss = ctx.enter_context(tc.tile_pool(name="pss", bufs=2, space="PSUM"))
            pss2 = ctx.enter_context(tc.tile_pool(name="pss2", bufs=2, space="PSUM"))
            pav = ctx.enter_context(tc.tile_pool(name="pav", bufs=2, space="PSUM"))

            # static tiles; wq is split per D-chunk into separate tiles so
            # the first projection matmuls only wait on the pieces they read
            # (Tile tracks dependencies at whole-tile granularity).
            wq_sb = [wpool.tile([P, FLOC], F32R, tag=f"wq{dc}", name=f"wq{dc}")
                     for dc in range(DC)]
            wk_sb = wpool.tile([P, DC, FLOC], F32R, tag="wk")
            wv_sb = wpool.tile([P, DC, FLOC], F32R, tag="wv")
            wo_sb = persist.tile([P, FC, D], F32R, tag="wo")
            mask_sb = persist.tile([P, 2 * P], F32, tag="mask")
            ones1 = persist.tile([1, DK], F32R, tag="ones1")

            kt_blocks = []
            v_blocks = []
            qt_blocks = []

            def phase1_chunks(sb):
                """QKV projections for seq block sb, as a list of emission
                chunks so they can be interleaved with attention work."""
                chunks = []

                state = {}

                def setup():
                    xt_t = [xtp.tile([P, 512], F32R, tag=f"xt{dc}", name=f"xt{dc}")
                            for dc in range(DC)]
                    for dc in range(DC):
                        if sb == 0:
                            # interleave the wq pieces so the first projection
                            # group streams right behind the DMA dispatches
                            nc.sync.dma_start(wq_sb[dc][:], wqT_a[:, dc, :])
                        nc.sync.dma_start(
                            xt_t[dc][:],
                            xT_a[:, dc, sb * 512:(sb + 1) * 512])
                    qt_t = qtp.tile([P, FC, 512], F32R, tag="qt")
                    kt_t = persist.tile([P, FC, 512], F32R, tag=f"kt{sb}")
                    v_t = persist.tile([P, 4, HLOC, DK + 1], F32R, tag=f"v{sb}")
                    qt_blocks.append(qt_t)
                    kt_blocks.append(kt_t)
                    v_blocks.append(v_t)
                    # ones column of V_aug (softmax denominators)
                    nc.sync.dma_start(
                        v_t[:, :, :, DK],
                        onesc.ap()[:, 0:32].bitcast(F32R)
                        .rearrange("p (a b) -> p a b", a=4),
                    )
                    state.update(xt=xt_t, qt=qt_t, kt=kt_t, v=v_t)

                chunks.append(setup)

                def q_chunk(fc):
                    def run():
                        ps_q = pss.tile([P, 512], F32, tag="s")
                        for dc in range(DC):
                            nc.tensor.matmul(
                                ps_q[:], wq_sb[dc][:, fc * P:(fc + 1) * P],
                                state["xt"][dc][:],
                                start=(dc == 0), stop=(dc == DC - 1),
                            )
                        nc.scalar.copy(state["qt"][:, fc, :], ps_q[:])
                    return run

                def k_chunk(fc):
                    def run():
                        ps_k = pss.tile([P, 512], F32, tag="s")
                        for dc in range(DC):
                            nc.tensor.matmul(
                                ps_k[:], wk_sb[:, dc, fc * P:(fc + 1) * P],
                                state["xt"][dc][:],
                                start=(dc == 0), stop=(dc == DC - 1),
                            )
                        nc.scalar.copy(state["kt"][:, fc, :], ps_k[:])
                    return run

                def v_chunk(sc):
                    def run():
                        ps_v = pss.tile([P, 512], F32, tag="s")
                        for dc in range(DC):
                            nc.tensor.matmul(
                                ps_v[:], state["xt"][dc][:, sc * P:(sc + 1) * P],
                                wv_sb[:, dc, :],
                                start=(dc == 0), stop=(dc == DC - 1),
                            )
                        nc.scalar.copy(
                            state["v"][:, sc, :, 0:DK],
                            ps_v[:].rearrange("p (h d) -> p h d", h=HLOC),
                        )
                    return run

                for fc in range(FC):
                    chunks.append(q_chunk(fc))
                for fc in range(FC):
                    chunks.append(k_chunk(fc))
                for sc in range(4):
                    chunks.append(v_chunk(sc))
                return chunks

            def phase1(sb):
                for ch in phase1_chunks(sb):
                    ch()

            def attention_chunks(qb):
                """Attention + output projection for query block qb, as
                emission chunks [pair0..pair3, outproj]."""
                qt_t = qt_blocks[qb]
                at_t = atp.tile([P, FC, 512], F32R, tag="at")
                last = 4 * qb + 3

                def pair_chunk(p):
                    def run():
                        do_pair(p)
                    return run

                def do_pair(p):  # head pair
                    ps_a0 = pav.tile([DK + 1, 512], F32, tag="av")
                    ps_a1 = pav.tile([DK + 1, 512], F32, tag="av")

                    def emit_av(e_pair, col0, kc):
                        v_t = v_blocks[kc // 4]
                        nc.tensor.matmul(
                            ps_a0[:, col0:], v_t[:, kc % 4, 2 * p, :],
                            e_pair[:, 0, col0:], start=(kc == 0), stop=(kc == last),
                        )
                        nc.tensor.matmul(
                            ps_a1[:, col0:], v_t[:, kc % 4, 2 * p + 1, :],
                            e_pair[:, 1, col0:], start=(kc == 0), stop=(kc == last),
                        )

                    pend = []  # AV two cells behind: hides the exp latency
                    for kc in range(last + 1):
                        r = kc - 4 * qb
                        col0 = P * r if r >= 0 else 0
                        mcol0 = col0
                        kt_t = kt_blocks[kc // 4]
                        ks = slice((kc % 4) * P, (kc % 4 + 1) * P)
                        ps_s = pss2.tile([P, 2, 512], F32, tag="s2")
                        nc.tensor.matmul(
                            ps_s[:, 0, mcol0:], kt_t[0:DK, p, ks],
                            qt_t[0:DK, p, mcol0:], start=True, stop=True,
                        )
                        nc.tensor.matmul(
                            ps_s[:, 1, mcol0:], kt_t[DK:P, p, ks],
                            qt_t[DK:P, p, mcol0:], start=True, stop=True,
                        )
                        if r >= 0:
                            mstart = mcol0 if r == 3 else col0
                            region = slice(mstart, col0 + P)
                            w = col0 + P - mstart
                            nc.vector.tensor_tensor(
                                ps_s[:, :, region],
                                ps_s[:, :, region],
                                mask_sb[:, 2 * P - w:]
                                .unsqueeze(1).to_broadcast([P, 2, w]),
                                mybir.AluOpType.add,
                            )
                        e_pair = ep.tile([P, 2, 512], F32R, tag="e")
                        nc.scalar.activation(
                            e_pair[:, :, mcol0:], ps_s[:, :, mcol0:],
                            mybir.ActivationFunctionType.Exp, scale=SCALE,
                        )
                        pend.append((e_pair, mcol0, kc))
                        if len(pend) > 5:
                            emit_av(*pend.pop(0))
                    for it in pend:
                        emit_av(*it)
                    # normalize: at[f, q] = a[f, q] / a[64, q], heads 2p, 2p+1
                    for half, ps_a in ((0, ps_a0), (1, ps_a1)):
                        inv_r = invp.tile([1, 512], F32R, tag="invr")
                        with nc.allow_low_precision(reason="fp32r matmul input"):
                            nc.vector.reciprocal(inv_r[:], ps_a[DK:DK + 1, :])
                        ps_rep = pss.tile([P, 512], F32, tag="s")
                        nc.tensor.matmul(ps_rep[0:DK, :], ones1[:], inv_r[:],
                                         start=True, stop=True)
                        rep = repp.tile([DK, 512], F32, tag="rep")
                        if half == 0:
                            nc.scalar.copy(rep[:], ps_rep[0:DK, :])
                        else:
                            nc.vector.tensor_copy(rep[:], ps_rep[0:DK, :])
                        nc.vector.tensor_tensor(
                            at_t[half * DK:(half + 1) * DK, p, :],
                            ps_a[0:DK, :], rep[:], mybir.AluOpType.mult,
                        )
                def outproj():
                    # output projection: outT[j, q] partial
                    for ob in range(OB):
                        ps_o = pss.tile([P, 512], F32, tag="s")
                        for fc in range(FC):
                            nc.tensor.matmul(
                                ps_o[:], wo_sb[:, fc, ob * P:(ob + 1) * P],
                                at_t[:, fc, :],
                                start=(fc == 0), stop=(fc == FC - 1),
                            )
                        o_t = outp.tile([P, 512], F32, tag="o")
                        nc.vector.tensor_copy(o_t[:], ps_o[:])
                        nc.sync.dma_start(
                            outT_a[:, ob, qb * 512:(qb + 1) * 512], o_t[:])

                return [pair_chunk(p) for p in range(FC)], outproj

            # Emission: phase1(0) with the late weights DMA'd behind the
            # first projection groups, then attention(qb) interleaved with
            # phase1(qb+1) so attention's ACT-bound stretches overlap the
            # projections' PE-bound stretches.
            ph = phase1_chunks(0)
            ph[0]()                      # xt(0) DMA right after wq
            nc.sync.dma_start(wk_sb[:], wkT_a)
            nc.sync.dma_start(wv_sb[:], wvT_a)
            for ch in ph[1:5]:
                ch()
            nc.sync.dma_start(wo_sb[:], woT_a)
            nc.sync.dma_start(mask_sb[:], maskb.ap())
            nc.sync.dma_start(ones1[:], onesc.ap()[0:1, 0:DK].bitcast(F32R))
            for ch in ph[5:]:
                ch()
            for qb in range(NB):
                att, opj = attention_chunks(qb)
                att = att + [opj]
                nxt = phase1_chunks(qb + 1) if qb < NB - 1 else []
                seq = []
                k = 0
                if nxt:
                    seq.append(nxt[0])
                    k = 1
                for i, pc in enumerate(att):
                    seq.append(pc)
                    for _ in range(3):
                        if k < len(nxt):
                            seq.append(nxt[k])
                            k += 1
                while k < len(nxt):
                    seq.append(nxt[k])
                    k += 1
                for ch in seq:
                    ch()


def _build():
    nc = bacc.Bacc("TRN2", target_bir_lowering=False, debug=False)
    _emit(nc)
    nc.compile()
    return nc


def _make_in_maps(x, W_q, W_k, W_v, W_o):
    onesc = np.ones((P, P), dtype=np.float32)
    kk = np.arange(P)[:, None]
    jj = np.arange(P)[None, :]
    band = np.where(kk <= jj, 0.0, NEG).astype(np.float32)
    # [128, 256]: first 128 cols fully masked (r=3 widened tiles), then the
    # triangular diagonal band
    maskb = np.concatenate(
        [np.full((P, P), NEG, np.float32), band], axis=1)
    in_maps = []
    for c in range(8):
        b, g = divmod(c, 2)
        cols = slice(g * FLOC, (g + 1) * FLOC)
        in_maps.append({
            "xT": np.ascontiguousarray(x[b].T),
            "wqT": np.ascontiguousarray(W_q[cols, :].T),
            "wkT": np.ascontiguousarray(W_k[cols, :].T),
            "wvT": np.ascontiguousarray(W_v[cols, :].T),
            "woT": np.ascontiguousarray(W_o[:, cols].T),
            "onesc": onesc,
            "maskb": maskb,
        })
    return in_maps


def kernel(x, W_q, W_k, W_v, W_o):
    x = np.asarray(x, dtype=np.float32)
    W_q = np.asarray(W_q, dtype=np.float32)
    W_k = np.asarray(W_k, dtype=np.float32)
    W_v = np.asarray(W_v, dtype=np.float32)
    W_o = np.asarray(W_o, dtype=np.float32)
    if "nc" not in _CACHE:
        _CACHE["nc"] = _build()
    nc = _CACHE["nc"]
    in_maps = _make_in_maps(x, W_q, W_k, W_v, W_o)
    res = bass_utils.run_bass_kernel_spmd(nc, in_maps, core_ids=list(range(8)))
    B = x.shape[0]
    out = np.empty((B, S, D), dtype=np.float32)
    for b in range(B):
        acc = res.results[2 * b]["outT"] + res.results[2 * b + 1]["outT"]
        out[b] = acc.T
    return out
